# revision 5
# baseline (speedup 1.0000x reference)
"""MinLSTM scan kernel for 8 Trainium2 NeuronCores (Bass/Tile).

Problem: h_{t} recurrence over T=2048 steps, B=16384 samples, S=10 states.
    gi = sigmoid(x*Wui^T + bui + h@Wus^T + bus)
    gf = sigmoid(x*Wfi^T + bfi + h@Wfs^T + bfs)
    nh = x*Wni^T + bni + h@Wns^T + bns
    h' = gi*nh + gf*h
    out = h_T @ Wr^T + br

Sharding: pure data-parallel over batch, 2048 samples per core.

Per-core layout: 2 pipeline groups x 1024 samples; each group packs its
samples into a [10 x 103] grid (10 partition-blocks x 103 columns, 6 pad
slots).  The recurrent state lives in SBUF "z" ring buffers of shape
[111, 103]: rows 0:100 = h (10 blocks x 10 states), rows 100:110 = x
(one row per block), row 110 = ones (bias).  Each step runs 3 block-
diagonal matmuls (lhsT [111,100]) producing u|f pre-activations in one
PSUM bank and nh in another, one ScalarE sigmoid over [100, 206], then
t1 = gi*nh (VectorE), t2 = gf*h (GpSimd), h' = t1 + t2 (VectorE) written
into the next ring slot.  x rows are DMA-prefetched from a host-packed,
pre-transposed xt tensor in DRAM.
"""

import os
import sys

import numpy as np

for _p in ("/opt/trn_rl_repo",):
    if os.path.isdir(_p) and _p not in sys.path:
        sys.path.insert(0, _p)

import concourse.bass as bass  # noqa: E402
import concourse.tile as tile  # noqa: E402
from concourse import bacc, mybir  # noqa: E402
from concourse.bass_utils import run_bass_kernel_spmd  # noqa: E402

F32 = mybir.dt.float32
AF = mybir.ActivationFunctionType
ALU = mybir.AluOpType

NCORES = 8
S = 10            # hidden states
NG = 2            # pipeline groups per core
BG = 1024         # samples per group
BC = NG * BG      # samples per core
R = 10            # sample blocks per group (partition packing)
NCOL = 103        # columns per group (R*NCOL = 1030 >= BG)
SLOTS = R * NCOL  # 1030
M = R * S         # 100 psum rows per gate
KZ = M + R + 1    # 111 z rows: h + x + ones
DRING = 8         # z ring depth (x prefetch distance)

# Segment length (compile-time steps per NEFF). 2047 = full unroll.
SEG_STEPS = int(os.environ.get("MINLSTM_SEG", "2047"))

PROFILE = False
TRACE_DIR = None
LAST_EXEC_NS = None

_prog_cache: dict[int, object] = {}


def _build_program(nsteps: int):
    """Build + compile the per-core Bass program for `nsteps` scan steps."""
    nc = bacc.Bacc(
        "TRN2",
        target_bir_lowering=False,
        debug=False,
        enable_asserts=False,
        num_devices=NCORES,
    )

    xt = nc.dram_tensor("xt", [nsteps, NG, R, NCOL], F32, kind="ExternalInput").ap()
    h_in = nc.dram_tensor("h_in", [NG, M, NCOL], F32, kind="ExternalInput").ap()
    wu_d = nc.dram_tensor("wu", [KZ, M], F32, kind="ExternalInput").ap()
    wf_d = nc.dram_tensor("wf", [KZ, M], F32, kind="ExternalInput").ap()
    wn_d = nc.dram_tensor("wn", [KZ, M], F32, kind="ExternalInput").ap()
    wr_d = nc.dram_tensor("wr", [M, R], F32, kind="ExternalInput").ap()
    br_d = nc.dram_tensor("brv", [R, 1], F32, kind="ExternalInput").ap()
    ones_d = nc.dram_tensor("onesrow", [1, NCOL], F32, kind="ExternalInput").ap()
    h_out = nc.dram_tensor("h_out", [NG, M, NCOL], F32, kind="ExternalOutput").ap()
    y_out = nc.dram_tensor("y", [NG, R, NCOL], F32, kind="ExternalOutput").ap()

    with tile.TileContext(nc) as tc:
        with (
            tc.tile_pool(name="w", bufs=1) as wp,
            tc.tile_pool(name="state", bufs=1) as sp,
            tc.tile_pool(name="work", bufs=1) as kp,
            tc.tile_pool(name="psum", bufs=1, space="PSUM") as pp,
        ):
            WU = wp.tile([KZ, M], F32, tag="WU")
            WF = wp.tile([KZ, M], F32, tag="WF")
            WN = wp.tile([KZ, M], F32, tag="WN")
            WR = wp.tile([M, R], F32, tag="WR")
            BR = wp.tile([R, 1], F32, tag="BR")
            nc.sync.dma_start(WU[:], wu_d[:])
            nc.sync.dma_start(WF[:], wf_d[:])
            nc.sync.dma_start(WN[:], wn_d[:])
            nc.sync.dma_start(WR[:], wr_d[:])
            nc.sync.dma_start(BR[:], br_d[:])

            Z = [
                [sp.tile([KZ, NCOL], F32, tag=f"z{g}_{d}", name=f"z{g}_{d}") for d in range(DRING)]
                for g in range(NG)
            ]
            GS = [
                [kp.tile([M, 2 * NCOL], F32, tag=f"gs{g}_{b}", name=f"gs{g}_{b}") for b in range(2)]
                for g in range(NG)
            ]
            T1 = [
                [kp.tile([M, NCOL], F32, tag=f"t1_{g}_{b}", name=f"t1_{g}_{b}") for b in range(2)]
                for g in range(NG)
            ]
            T2 = [
                [kp.tile([M, NCOL], F32, tag=f"t2_{g}_{b}", name=f"t2_{g}_{b}") for b in range(2)]
                for g in range(NG)
            ]
            UF = [
                [pp.tile([M, 2 * NCOL], F32, tag=f"uf{g}_{b}", name=f"uf{g}_{b}") for b in range(2)]
                for g in range(NG)
            ]
            NB = [
                [pp.tile([M, NCOL], F32, tag=f"nb{g}_{b}", name=f"nb{g}_{b}") for b in range(2)]
                for g in range(NG)
            ]

            # --- init: h state, ones rows, x prefill ---
            for g in range(NG):
                nc.sync.dma_start(Z[g][0][0:M, :], h_in[g])
                for d in range(DRING):
                    nc.sync.dma_start(Z[g][d][M + R : KZ, :], ones_d[:])
                for i in range(min(DRING - 1, nsteps)):
                    nc.sync.dma_start(Z[g][i][M : M + R, :], xt[i, g])

            # --- main scan ---
            for i in range(nsteps):
                b = i & 1
                for g in range(NG):
                    zi = Z[g][i % DRING]
                    zn = Z[g][(i + 1) % DRING]
                    uf = UF[g][b]
                    nb = NB[g][b]
                    gs = GS[g][b]
                    t1 = T1[g][b]
                    t2 = T2[g][b]
                    nc.tensor.matmul(uf[:, 0:NCOL], WU[:], zi[:])
                    nc.tensor.matmul(uf[:, NCOL : 2 * NCOL], WF[:], zi[:])
                    nc.tensor.matmul(nb[:], WN[:], zi[:])
                    nc.scalar.activation(gs[:], uf[:], AF.Sigmoid)
                    nc.vector.tensor_mul(t1[:], gs[:, 0:NCOL], nb[:])
                    nc.gpsimd.tensor_tensor(
                        t2[:], gs[:, NCOL : 2 * NCOL], zi[0:M, :], ALU.mult
                    )
                    nc.vector.tensor_add(zn[0:M, :], t1[:], t2[:])
                    ip = i + DRING - 1
                    if ip < nsteps:
                        nc.sync.dma_start(
                            Z[g][ip % DRING][M : M + R, :], xt[ip, g]
                        )

            # --- epilogue: h_out + y = h @ Wr^T + br ---
            for g in range(NG):
                zf = Z[g][nsteps % DRING]
                nc.sync.dma_start(h_out[g], zf[0:M, :])
                yp = UF[g][0][0:R, 0:NCOL]
                nc.tensor.matmul(yp, WR[:], zf[0:M, :])
                ys = T1[g][0][0:R, :]
                nc.vector.tensor_scalar_add(ys, yp, BR[:, 0:1])
                nc.sync.dma_start(y_out[g], ys)

    nc.compile()
    return nc


def _get_program(nsteps: int):
    if nsteps not in _prog_cache:
        _prog_cache[nsteps] = _build_program(nsteps)
    return _prog_cache[nsteps]


def _pack_gate_w(Ws: np.ndarray, Wi: np.ndarray, bias: np.ndarray) -> np.ndarray:
    """lhsT [KZ, M] for one gate: block-diag Ws^T, x rows Wi, ones row bias."""
    w = np.zeros((KZ, M), np.float32)
    for r in range(R):
        w[r * S : (r + 1) * S, r * S : (r + 1) * S] = Ws.T
        w[M + r, r * S : (r + 1) * S] = Wi[:, 0]
    w[M + R, :] = np.tile(bias, R)
    return w


def kernel(
    X, Wui, bui, Wus, bus, Wfi, bfi, Wfs, bfs, Wni, bni, Wns, bns, Wr, br
) -> np.ndarray:
    global LAST_EXEC_NS
    X = np.asarray(X, np.float32)
    Bfull, T = X.shape
    assert Bfull == NCORES * BC, f"expected B={NCORES * BC}, got {Bfull}"
    nsteps_total = T - 1

    f = lambda a: np.asarray(a, np.float32)
    wu = _pack_gate_w(f(Wus), f(Wui), f(bui) + f(bus))
    wf = _pack_gate_w(f(Wfs), f(Wfi), f(bfi) + f(bfs))
    wn = _pack_gate_w(f(Wns), f(Wni), f(bni) + f(bns))
    wr = np.zeros((M, R), np.float32)
    for r in range(R):
        wr[r * S : (r + 1) * S, r] = f(Wr)[0]
    brv = np.full((R, 1), f(br)[0], np.float32)

    # --- host packing: per-core transposed x and initial h ---
    xt_cores = []
    h_cores = []
    for c in range(NCORES):
        Xc = X[c * BC : (c + 1) * BC]  # [2048, T]
        xt_c = np.zeros((nsteps_total, NG, R, NCOL), np.float32)
        h0_c = np.zeros((NG, M, NCOL), np.float32)
        for g in range(NG):
            P = np.zeros((SLOTS, T), np.float32)
            P[:BG] = Xc[g * BG : (g + 1) * BG]
            Pr = P.reshape(R, NCOL, T)
            xt_c[:, g] = np.ascontiguousarray(Pr[:, :, 1:].transpose(2, 0, 1))
            h0g = np.zeros((R, S, NCOL), np.float32)
            h0g[:, 0, :] = Pr[:, :, 0]
            h0_c[g] = h0g.reshape(M, NCOL)
        xt_cores.append(xt_c)
        h_cores.append(h0_c)

    # --- segment schedule ---
    segs = []
    done = 0
    while done < nsteps_total:
        n = min(SEG_STEPS, nsteps_total - done)
        segs.append((done, n))
        done += n

    LAST_EXEC_NS = 0
    y_last = None
    for lo, n in segs:
        prog = _get_program(n)
        in_maps = []
        for c in range(NCORES):
            in_maps.append(
                {
                    "xt": np.ascontiguousarray(xt_cores[c][lo : lo + n]),
                    "h_in": h_cores[c],
                    "wu": wu,
                    "wf": wf,
                    "wn": wn,
                    "wr": wr,
                    "brv": brv,
                    "onesrow": np.ones((1, NCOL), np.float32),
                }
            )
        kw = {}
        if PROFILE and TRACE_DIR:
            kw["tmpdir"] = TRACE_DIR
        res = run_bass_kernel_spmd(
            prog, in_maps, core_ids=list(range(NCORES)), trace=PROFILE, **kw
        )
        if res.exec_time_ns is not None:
            LAST_EXEC_NS += res.exec_time_ns
        h_cores = [np.asarray(res.results[c]["h_out"]) for c in range(NCORES)]
        y_last = [np.asarray(res.results[c]["y"]) for c in range(NCORES)]

    out = np.zeros((Bfull, 1), np.float32)
    for c in range(NCORES):
        for g in range(NG):
            flat = y_last[c][g].reshape(SLOTS)[:BG]
            out[c * BC + g * BG : c * BC + (g + 1) * BG, 0] = flat
    return out
